# revision 19
# baseline (speedup 1.0000x reference)
"""Edge-parallel NNConv (CellNet) kernel for 8 Trainium2 NeuronCores.

Strategy
--------
Nodes are assigned to (core, partition p, block r) slots by LPT bin-packing
on degree, so each of the 8*49 blocks of 128 nodes has a near-equal edge
count; every edge lives on the bin of its destination node.  Per layer,
per core, per node-block r (tpb edge tiles of 128 edges, host-padded):

  1. z-matmul (PE): the edge network pre-activation
        z[e,:] = emb[et]*(ef@Wh+bh) + (ef@Wg+bg)
     refactored as z[e,:] = SO[:,e]^T @ C123 with
        SO   = [onehot(et)*ef1 ; onehot(et)*ef2 ; onehot(et)]   (75 x E)
        C123 = [emb*Wh0+Wg0 ; emb*Wh1+Wg1 ; emb*bh+bg]          (75 x d)
     SO is host-built input re-formatting (bf16, SBUF-resident); C123 is a
     tiny host-folded weight.  One matmul per 128-edge tile -> z in PSUM.
  2. fused relu*x (DVE): p[e,(o,i)] = max(z,0)*x_src[e,i] via one
     scalar_tensor_tensor per tile (x broadcast with a step-0 AP), bf16 out.
  3. segment-sum via PE: agg_wide[m,(o,i)] += S^T[e,m] p[e,(o,i)] where S^T
     is the host-built dst one-hot (bf16, streamed).  All of a block's tiles
     accumulate into one PSUM tile; a rank-1 (deg x bias) matmul seeds the
     bias so the later divide yields agg + bias exactly.
  4. node stage (batched over the 49 blocks): reduce over i (N-sized, not
     E-sized), multiply by host 1/deg, add the root term via per-block PE
     matmuls on transposed activations, relu (h kept in bf16).
  5. AllGather of the per-core chunk gives every core full h for the next
     layer's x_src gathers (indirect DMA by global row).  PE transposes own
     blocks for the next root term.
  6. Final layer: masked per-graph mean pooling via PE matmuls against a
     host one-hot, a [8,16] AllReduce, and a 1/count scale.

x_src for layer 1 is host-gathered (x is an input); layers 2-3 gather h
rows on-device with indirect DMA (cheap: ~0.7ns/row descriptor-gen).
"""

import math

import numpy as np
import ml_dtypes

N_CORES = 8
LAYER_DIMS = [(16, 10), (10, 10), (10, 16)]
N_ET = 25
K_SO = 3 * N_ET
PSUM_BANK_F32 = 512


class Cfg:
    def __init__(self, n_nodes, n_edges, npc, tpb):
        self.n_nodes = n_nodes
        self.n_edges = n_edges
        self.npc = npc                       # nodes per core
        self.nblk = math.ceil(npc / 128)     # node blocks per core
        self.npad = 128 * self.nblk
        self.tpb = tpb                       # edge tiles per block (uniform)
        self.epad = self.nblk * tpb * 128    # padded edges per core
        self.ntiles = self.nblk * tpb


_BUILD_CACHE = {}


def build_nc(cfg: Cfg, debug=False):
    import concourse.bacc as bacc
    import concourse.bass as bass
    import concourse.mybir as mybir
    import concourse.tile as tile
    from concourse.masks import make_identity

    f32 = mybir.dt.float32
    bf16 = mybir.dt.bfloat16
    i32 = mybir.dt.int32
    AX = mybir.AxisListType
    OP = mybir.AluOpType

    nc = bacc.Bacc("TRN2", target_bir_lowering=False, debug=False,
                   num_devices=N_CORES)

    nblk, npad, epad, tpb = cfg.nblk, cfg.npad, cfg.epad, cfg.tpb

    # ---- kernel I/O ------------------------------------------------------
    so_d = nc.dram_tensor("so", [K_SO, epad], bf16, kind="ExternalInput")
    st_d = nc.dram_tensor("st", [epad, 128], bf16, kind="ExternalInput")
    xsrc1_d = nc.dram_tensor("xsrc1", [nblk, 128, tpb * 16], bf16,
                             kind="ExternalInput")
    goff_d = nc.dram_tensor("goff", [nblk, 128, tpb], i32,
                            kind="ExternalInput")
    c123_d = [nc.dram_tensor(f"c123_{l}", [K_SO, ic * oc], bf16,
                             kind="ExternalInput")
              for l, (ic, oc) in enumerate(LAYER_DIMS)]
    rb_d = [nc.dram_tensor(f"rb_{l}", [ic, oc], f32, kind="ExternalInput")
            for l, (ic, oc) in enumerate(LAYER_DIMS)]
    bw_d = [nc.dram_tensor(f"bw_{l}", [1, ic * oc], bf16,
                           kind="ExternalInput")
            for l, (ic, oc) in enumerate(LAYER_DIMS)]
    xt1_d = nc.dram_tensor("xt1", [16, npad], f32, kind="ExternalInput")
    degb_d = nc.dram_tensor("degb", [1, npad], bf16, kind="ExternalInput")
    degr_d = nc.dram_tensor("degr", [128, nblk], f32, kind="ExternalInput")
    ppool_d = nc.dram_tensor("ppool", [128, nblk * 8], bf16,
                             kind="ExternalInput")
    cntr_d = nc.dram_tensor("cntr", [8, 1], f32, kind="ExternalInput")
    out_d = nc.dram_tensor("out", [8, 16], f32, kind="ExternalOutput")
    if debug:
        dbg_h1 = nc.dram_tensor("dbg_h1", [128, cfg.nblk * 10], bf16,
                                kind="ExternalOutput")
        dbg_as1 = nc.dram_tensor("dbg_as1", [128, cfg.nblk * 10], f32,
                                 kind="ExternalOutput")
        dbg_rt1 = nc.dram_tensor("dbg_rt1", [128, cfg.nblk * 10], f32,
                                 kind="ExternalOutput")
        dbg_h2 = nc.dram_tensor("dbg_h2", [128, cfg.nblk * 10], bf16,
                                kind="ExternalOutput")
        dbg_rt2 = nc.dram_tensor("dbg_rt2", [128, cfg.nblk * 10], f32,
                                 kind="ExternalOutput")
        dbg_as2 = nc.dram_tensor("dbg_as2", [128, cfg.nblk * 10], f32,
                                 kind="ExternalOutput")
        dbg_xg = nc.dram_tensor("dbg_xg", [128, cfg.tpb * 10], bf16,
                                kind="ExternalOutput")
        dbg_hf = nc.dram_tensor("dbg_hf", [N_CORES, 128, cfg.nblk * 10],
                                bf16, kind="ExternalOutput")

    groups = [list(range(N_CORES))]

    with tile.TileContext(nc) as tc:
        with (
            tc.tile_pool(name="res", bufs=1) as res,
            tc.tile_pool(name="st", bufs=3) as st_pool,
            tc.tile_pool(name="xg", bufs=3) as xg_pool,
            tc.tile_pool(name="pb", bufs=3) as pb_pool,
            tc.tile_pool(name="node", bufs=1) as node_pool,
            tc.tile_pool(name="small", bufs=2) as small_pool,
            tc.tile_pool(name="zp", bufs=2, space="PSUM") as zp_pool,
            tc.tile_pool(name="ag", bufs=2, space="PSUM") as ag_pool,
            tc.tile_pool(name="rt", bufs=1, space="PSUM") as rt_pool,
            tc.tile_pool(name="tp", bufs=2, space="PSUM") as tp_pool,
            tc.tile_pool(name="dram", bufs=1, space="DRAM") as dram,
        ):
            # ---- residents ---------------------------------------------
            so_sb = res.tile([K_SO, epad], bf16)
            nc.sync.dma_start(out=so_sb[:], in_=so_d.ap())
            c123_sb = []
            for l in range(3):
                t = res.tile([K_SO, LAYER_DIMS[l][0] * LAYER_DIMS[l][1]],
                             bf16, tag=f"c123_{l}", name=f"c123s{l}")
                nc.sync.dma_start(out=t[:], in_=c123_d[l].ap())
                c123_sb.append(t)
            rb_sb = []
            bw_sb = []
            for l in range(3):
                ic, oc = LAYER_DIMS[l]
                t = res.tile([ic, oc], f32, tag=f"rb_{l}", name=f"rbs{l}")
                nc.sync.dma_start(out=t[:], in_=rb_d[l].ap())
                rb_sb.append(t)
                t2 = res.tile([1, ic * oc], bf16, tag=f"bw_{l}",
                              name=f"bws{l}")
                nc.sync.dma_start(out=t2[:], in_=bw_d[l].ap())
                bw_sb.append(t2)
            xt1_sb = res.tile([16, npad], f32)
            nc.sync.dma_start(out=xt1_sb[:], in_=xt1_d.ap())
            degb_sb = res.tile([1, npad], bf16)
            nc.sync.dma_start(out=degb_sb[:], in_=degb_d.ap())
            degr_sb = res.tile([128, nblk], f32)
            nc.sync.dma_start(out=degr_sb[:], in_=degr_d.ap())
            ppool_sb = res.tile([128, nblk * 8], bf16)
            nc.sync.dma_start(out=ppool_sb[:], in_=ppool_d.ap())
            cntr_sb = res.tile([8, 1], f32)
            nc.sync.dma_start(out=cntr_sb[:], in_=cntr_d.ap())
            ident = res.tile([128, 128], bf16)
            make_identity(nc, ident[:])
            xt_all = res.tile([10, npad], f32)

            # ---- DRAM scratch ------------------------------------------
            hc = [dram.tile([128, nblk * 10], bf16, tag=f"hc{l}",
                            name=f"hc{l}") for l in range(2)]
            hf = [dram.tile([N_CORES, 128, nblk * 10], bf16, tag=f"hf{l}",
                            name=f"hf{l}", addr_space="Shared")
                  for l in range(2)]
            hfl = [dram.tile([N_CORES, 128, nblk * 10], bf16, tag=f"hfl{l}",
                             name=f"hfl{l}") for l in range(2)]
            pool_in = dram.tile([8, 16], f32)
            pool_out = dram.tile([8, 16], f32, addr_space="Shared")

            for l in range(3):
                ic, oc = LAYER_DIMS[l]
                d = ic * oc
                gmax = PSUM_BANK_F32 // d

                asum = node_pool.tile([128, nblk * oc], f32, tag="asum")
                # ---- edge phase: one batch per node block --------------
                for r in range(nblk):
                    xg = xg_pool.tile([128, tpb, ic], bf16, tag="xg")
                    if l == 0:
                        nc.sync.dma_start(out=xg[:], in_=xsrc1_d.ap()[r])
                    else:
                        xo = xg_pool.tile([128, tpb], i32, tag="xo")
                        nc.sync.dma_start(out=xo[:], in_=goff_d.ap()[r])
                        for t in range(tpb):
                            nc.gpsimd.indirect_dma_start(
                                out=xg[:, t, :].bitcast(f32),
                                out_offset=None,
                                in_=hfl[l - 1][:].rearrange(
                                    "c p (q h) -> (c p q) h",
                                    h=10).bitcast(f32),
                                in_offset=bass.IndirectOffsetOnAxis(
                                    ap=xo[:, t:t + 1], axis=0),
                            )
                    stt = st_pool.tile([128, tpb, 128], bf16, tag="st")
                    nc.sync.dma_start(
                        out=stt[:],
                        in_=st_d.ap()[r * tpb * 128:(r + 1) * tpb * 128, :]
                        .rearrange("(t p) m -> p t m", p=128))
                    if debug and l == 1 and r == 0:
                        nc.sync.dma_start(
                            out=dbg_xg.ap(),
                            in_=xg[:].rearrange("p t i -> p (t i)"))
                    pbuf = pb_pool.tile([128, tpb * d], bf16, tag="pb")
                    t0 = 0
                    while t0 < tpb:
                        g = min(gmax, tpb - t0)
                        zp = zp_pool.tile([128, PSUM_BANK_F32], f32,
                                          tag="zp")
                        for j in range(g):
                            tl = r * tpb + t0 + j
                            nc.tensor.matmul(
                                out=zp[:, j * d:(j + 1) * d],
                                lhsT=so_sb[:, tl * 128:(tl + 1) * 128],
                                rhs=c123_sb[l][:],
                                start=True, stop=True)
                        for j in range(g):
                            t = t0 + j
                            nc.vector.scalar_tensor_tensor(
                                out=pbuf[:, t * d:(t + 1) * d].rearrange(
                                    "p (o i) -> p o i", o=oc, i=ic),
                                in0=zp[:, j * d:(j + 1) * d].rearrange(
                                    "p (o i) -> p o i", o=oc, i=ic),
                                scalar=0.0,
                                in1=xg[:, t:t + 1, :].to_broadcast(
                                    [128, oc, ic]),
                                op0=OP.max, op1=OP.mult)
                        t0 += g
                    # segment-sum into PSUM: bias seed + tpb one-hot matmuls
                    agp = ag_pool.tile([128, d], f32, tag="ag")
                    nc.tensor.matmul(
                        out=agp[:],
                        lhsT=degb_sb[:, r * 128:(r + 1) * 128],
                        rhs=bw_sb[l][:],
                        start=True, stop=False)
                    for t in range(tpb):
                        nc.tensor.matmul(
                            out=agp[:],
                            lhsT=stt[:, t, :],
                            rhs=pbuf[:, t * d:(t + 1) * d],
                            start=False, stop=(t == tpb - 1))
                    nc.vector.reduce_sum(
                        out=asum[:, r * oc:(r + 1) * oc],
                        in_=agp[:].rearrange("p (o i) -> p o i",
                                             o=oc, i=ic),
                        axis=AX.X)

                # ---- node phase (batched over blocks) ------------------
                agg = node_pool.tile([128, nblk * oc], f32, tag="agg")
                nc.vector.tensor_tensor(
                    out=agg[:].rearrange("p (q o) -> p q o", o=oc),
                    in0=asum[:].rearrange("p (q o) -> p q o", o=oc),
                    in1=degr_sb[:].unsqueeze(2).to_broadcast(
                        [128, nblk, oc]),
                    op=OP.mult)
                rt = rt_pool.tile([128, nblk * 16], f32, tag="rt")
                xt = xt1_sb if l == 0 else xt_all
                kr = 16 if l == 0 else 10
                for r in range(nblk):
                    nc.tensor.matmul(
                        out=rt[:, r * oc:(r + 1) * oc],
                        lhsT=xt[0:kr, r * 128:(r + 1) * 128],
                        rhs=rb_sb[l][:],
                        start=True, stop=True)
                hsum = node_pool.tile([128, nblk * oc], f32, tag="hsum")
                nc.vector.scalar_tensor_tensor(
                    out=hsum[:], in0=rt[:, :nblk * oc], scalar=0.0,
                    in1=agg[:], op0=OP.add, op1=OP.add)
                hrel = node_pool.tile([128, nblk * oc], bf16, tag="hrel")
                nc.vector.tensor_scalar_max(out=hrel[:], in0=hsum[:],
                                            scalar1=0.0)

                if debug and l == 1:
                    nc.sync.dma_start(out=dbg_h2.ap(), in_=hrel[:])
                    nc.sync.dma_start(out=dbg_as2.ap(), in_=asum[:])
                    hsf2 = node_pool.tile([128, nblk * oc], f32, tag="hsf2")
                    nc.scalar.copy(out=hsf2[:], in_=rt[:, :nblk * oc])
                    nc.sync.dma_start(out=dbg_rt2.ap(), in_=hsf2[:])
                if debug and l == 0:
                    nc.sync.dma_start(out=dbg_h1.ap(), in_=hrel[:])
                    nc.sync.dma_start(out=dbg_as1.ap(), in_=asum[:])
                    hsf = node_pool.tile([128, nblk * oc], f32, tag="hsf")
                    nc.scalar.copy(out=hsf[:], in_=rt[:, :nblk * oc])
                    nc.sync.dma_start(out=dbg_rt1.ap(), in_=hsf[:])
                if l < 2:
                    nc.sync.dma_start(out=hc[l][:], in_=hrel[:])
                    nc.gpsimd.collective_compute(
                        "AllGather", OP.bypass, replica_groups=groups,
                        ins=[hc[l][:]], outs=[hf[l][:]])
                    nc.sync.dma_start(out=hfl[l][:], in_=hf[l][:])
                    if debug and l == 0:
                        nc.sync.dma_start(out=dbg_hf.ap(), in_=hfl[l][:])
                    for r in range(nblk):
                        tp = tp_pool.tile([16, 128], bf16, tag="tp")
                        nc.tensor.transpose(
                            out=tp[0:oc, :],
                            in_=hrel[:, r * oc:(r + 1) * oc],
                            identity=ident[:])
                        nc.scalar.copy(
                            out=xt_all[0:oc, r * 128:(r + 1) * 128],
                            in_=tp[0:oc, :])
                else:
                    plp = tp_pool.tile([8, 16], f32, tag="tp",
                                       name="plp")
                    for r in range(nblk):
                        nc.tensor.matmul(
                            out=plp[:],
                            lhsT=ppool_sb[:, r * 8:(r + 1) * 8],
                            rhs=hrel[:, r * oc:(r + 1) * oc],
                            start=(r == 0), stop=(r == nblk - 1))
                    pool_sb = small_pool.tile([8, 16], f32, tag="pool_sb")
                    nc.scalar.copy(out=pool_sb[:], in_=plp[:])
                    nc.sync.dma_start(out=pool_in[:], in_=pool_sb[:])
                    nc.gpsimd.collective_compute(
                        "AllReduce", OP.add, replica_groups=groups,
                        ins=[pool_in[:]], outs=[pool_out[:]])
                    pool2 = small_pool.tile([8, 16], f32, tag="pool2")
                    nc.sync.dma_start(out=pool2[:], in_=pool_out[:])
                    outp = small_pool.tile([8, 16], f32, tag="outp")
                    nc.vector.tensor_scalar_mul(out=outp[:], in0=pool2[:],
                                                scalar1=cntr_sb[:, 0:1])
                    nc.sync.dma_start(out=out_d.ap(), in_=outp[:])

    nc.compile()
    return nc


# --------------------------------------------------------------------------
# host-side preparation
# --------------------------------------------------------------------------

def _assign_nodes(deg, n, npc, nblk):
    """LPT bin-packing: nodes -> (core, p, r) slots balancing per-block edge
    load with <=128 nodes per block.  Returns (core, p, r) arrays and the
    max block load."""
    import heapq

    nbins = N_CORES * nblk
    order = np.argsort(-deg, kind="stable")
    heap = [(0, b) for b in range(nbins)]  # (load, bin)
    heapq.heapify(heap)
    counts = np.zeros(nbins, np.int64)
    bin_of = np.zeros(n, np.int64)
    slot_of = np.zeros(n, np.int64)
    spill = []
    for node in order:
        load, b = heapq.heappop(heap)
        bin_of[node] = b
        slot_of[node] = counts[b]
        counts[b] += 1
        load += int(deg[node])
        if counts[b] < 128:
            heapq.heappush(heap, (load, b))
        else:
            spill.append((load, b))
    maxload = max([l for l, _ in heap] + [l for l, _ in spill])
    core = bin_of // nblk
    r = bin_of % nblk
    p = slot_of
    return core, p, r, maxload


def host_prep(cfg: Cfg, inputs: dict):
    np_f32 = np.float32
    bf = ml_dtypes.bfloat16
    x = np.asarray(inputs["x"], np_f32)
    ef = np.asarray(inputs["edge_feat"], np_f32)
    et = np.asarray(inputs["edge_type"]).astype(np.int64)
    src = np.asarray(inputs["edge_src"]).astype(np.int64)
    dst = np.asarray(inputs["edge_dst"]).astype(np.int64)
    cell = np.asarray(inputs["cell_type"]).astype(np.int64)
    bids = np.asarray(inputs["batch_ids"]).astype(np.int64)

    n = cfg.n_nodes
    npc, npad, nblk, tpb = cfg.npc, cfg.npad, cfg.nblk, cfg.tpb
    epad = cfg.epad

    deg = np.bincount(dst, minlength=n).astype(np.int64)
    ncore, npart, nr = inputs["_node_core"], inputs["_node_p"], inputs["_node_r"]

    # folded weights per layer (o-major columns)
    c123, rbs, bws = [], [], []
    for l, (ic, oc) in enumerate(LAYER_DIMS):
        i = l + 1
        emb = np.asarray(inputs[f"emb{i}"], np_f32)
        wh = np.asarray(inputs[f"wh{i}"], np_f32)
        bh = np.asarray(inputs[f"bh{i}"], np_f32)
        wg = np.asarray(inputs[f"wg{i}"], np_f32)
        bg = np.asarray(inputs[f"bg{i}"], np_f32)
        c1 = emb * wh[0][None, :] + wg[0][None, :]
        c2 = emb * wh[1][None, :] + wg[1][None, :]
        c3 = emb * bh[None, :] + bg[None, :]
        m = np.concatenate([c1, c2, c3], axis=0)
        j = np.arange(ic * oc)
        o_, i_ = j // ic, j % ic
        c123.append(np.ascontiguousarray(m[:, i_ * oc + o_]).astype(bf))
        rbs.append(np.asarray(inputs[f"root{i}"], np_f32))
        b = np.asarray(inputs[f"bias{i}"], np_f32)
        bwide = np.zeros(ic * oc, np_f32)
        bwide[np.arange(oc) * ic] = b  # column o*ic + i=0 gets bias[o]
        bws.append(bwide.reshape(1, ic * oc).astype(bf))

    # edge -> (core, block, partition-of-dst)
    e_core = ncore[dst]
    e_r = nr[dst]
    e_p = npart[dst]

    # global h row of src (h layout: row = p*nblk + r per core chunk)
    src_row = (ncore[src] * npad + npart[src] * nblk + nr[src]).astype(np.int64)

    gate = (cell == 1)
    cnt = np.bincount(bids[gate], minlength=8).astype(np_f32)
    cntr = (1.0 / np.maximum(cnt, 1.0)).reshape(8, 1).astype(np_f32)

    in_maps = []
    for c in range(N_CORES):
        esel = np.where(e_core == c)[0]
        # order edges by block, pad each block to tpb*128 slots
        blk = e_r[esel]
        order = np.argsort(blk, kind="stable")
        esel = esel[order]
        blk = blk[order]
        bc = np.bincount(blk, minlength=nblk)
        assert bc.max() <= tpb * 128, (bc.max(), tpb * 128)
        # slot index for each edge
        startslot = np.zeros(nblk, np.int64)
        startslot[:] = np.arange(nblk) * tpb * 128
        within = np.arange(len(esel)) - np.repeat(
            np.concatenate([[0], np.cumsum(bc)[:-1]]), bc)
        slots = np.repeat(startslot, bc) + within

        et_c = et[esel]
        ef_c = ef[esel]

        so = np.zeros((K_SO, epad), np_f32)
        so[et_c, slots] = ef_c[:, 0]
        so[N_ET + et_c, slots] = ef_c[:, 1]
        so[2 * N_ET + et_c, slots] = 1.0

        st = np.zeros((epad, 128), np_f32)
        st[slots, e_p[esel]] = 1.0

        xs = np.zeros((epad, 16), np_f32)
        xs[slots, :] = x[src[esel]]
        xs = xs.reshape(nblk, tpb, 128, 16).transpose(0, 2, 1, 3)
        xs = np.ascontiguousarray(xs).reshape(nblk, 128, tpb * 16)

        gr = np.zeros(epad, np.int64)
        gr[slots] = src_row[esel]
        goff = gr.reshape(nblk, tpb, 128).transpose(0, 2, 1)  # [r, p, t]
        goff = np.ascontiguousarray(goff).astype(np.int32)

        own = np.where(ncore == c)[0]
        ell = npart[own] * nblk + nr[own]

        xt1 = np.zeros((16, npad), np_f32)  # col r*128+p = node (p,r)
        xt1[:, nr[own] * 128 + npart[own]] = x[own].T

        degb = np.zeros((1, npad), np_f32)
        degb[0, nr[own] * 128 + npart[own]] = deg[own]

        degr = np.zeros(npad, np_f32)
        degr[ell] = 1.0 / np.maximum(deg[own], 1.0)
        degr = degr.reshape(128, nblk)

        pp = np.zeros((npad, 8), np_f32)
        g = gate[own]
        pp[ell[g], bids[own][g]] = 1.0
        pp = pp.reshape(128, nblk * 8)

        in_maps.append({
            "so": so.astype(bf),
            "st": st.astype(bf),
            "xsrc1": xs.astype(bf),
            "goff": goff,
            "c123_0": c123[0], "c123_1": c123[1], "c123_2": c123[2],
            "rb_0": rbs[0], "rb_1": rbs[1], "rb_2": rbs[2],
            "bw_0": bws[0], "bw_1": bws[1], "bw_2": bws[2],
            "xt1": xt1,
            "degb": degb.astype(bf),
            "degr": degr.astype(np_f32),
            "ppool": pp.astype(bf),
            "cntr": cntr,
        })
    return in_maps


def prepare(inputs: dict, n_nodes, n_edges, npc):
    """Compute node assignment + cfg (tpb depends on balance)."""
    dst = np.asarray(inputs["edge_dst"]).astype(np.int64)
    deg = np.bincount(dst, minlength=n_nodes).astype(np.int64)
    nblk = math.ceil(npc / 128)
    core, p, r, maxload = _assign_nodes(deg, n_nodes, npc, nblk)
    tpb = max(2, math.ceil(maxload / 128))
    cfg = Cfg(n_nodes, n_edges, npc, tpb)
    inputs = dict(inputs)
    inputs["_node_core"] = core
    inputs["_node_p"] = p
    inputs["_node_r"] = r
    return cfg, inputs


def kernel(**inputs) -> np.ndarray:
    from concourse.bass_utils import run_bass_kernel_spmd

    cfg, inputs2 = prepare(inputs, 50000, 250000, 6250)
    key = ("full", cfg.tpb)
    if key not in _BUILD_CACHE:
        _BUILD_CACHE[key] = build_nc(cfg)
    nc = _BUILD_CACHE[key]
    in_maps = host_prep(cfg, inputs2)
    res = run_bass_kernel_spmd(nc, in_maps, list(range(N_CORES)))
    return np.asarray(res.results[0]["out"], np.float32)


# revision 20
# speedup vs baseline: 1.0002x; 1.0002x over previous
"""Edge-parallel NNConv (CellNet) kernel for 8 Trainium2 NeuronCores.

Strategy
--------
Nodes are assigned to (core, partition p, block r) slots by LPT bin-packing
on degree, so each of the 8*49 blocks of 128 nodes has a near-equal edge
count; every edge lives on the bin of its destination node.  Per layer,
per core, per node-block r (tpb edge tiles of 128 edges, host-padded):

  1. z-matmul (PE): the edge network pre-activation
        z[e,:] = emb[et]*(ef@Wh+bh) + (ef@Wg+bg)
     refactored as z[e,:] = SO[:,e]^T @ C123 with
        SO   = [onehot(et)*ef1 ; onehot(et)*ef2 ; onehot(et)]   (75 x E)
        C123 = [emb*Wh0+Wg0 ; emb*Wh1+Wg1 ; emb*bh+bg]          (75 x d)
     SO is host-built input re-formatting (bf16, SBUF-resident); C123 is a
     tiny host-folded weight.  One matmul per 128-edge tile -> z in PSUM.
  2. fused relu*x (DVE): p[e,(o,i)] = max(z,0)*x_src[e,i] via one
     scalar_tensor_tensor per tile (x broadcast with a step-0 AP), bf16 out.
  3. segment-sum via PE: agg_wide[m,(o,i)] += S^T[e,m] p[e,(o,i)] where S^T
     is the host-built dst one-hot (bf16, streamed).  All of a block's tiles
     accumulate into one PSUM tile; a rank-1 (deg x bias) matmul seeds the
     bias so the later divide yields agg + bias exactly.
  4. node stage (batched over the 49 blocks): reduce over i (N-sized, not
     E-sized), multiply by host 1/deg, add the root term via per-block PE
     matmuls on transposed activations, relu (h kept in bf16).
  5. AllGather of the per-core chunk gives every core full h for the next
     layer's x_src gathers (indirect DMA by global row).  PE transposes own
     blocks for the next root term.
  6. Final layer: masked per-graph mean pooling via PE matmuls against a
     host one-hot, a [8,16] AllReduce, and a 1/count scale.

x_src for layer 1 is host-gathered (x is an input); layers 2-3 gather h
rows on-device with indirect DMA (cheap: ~0.7ns/row descriptor-gen).
"""

import math

import numpy as np
import ml_dtypes

N_CORES = 8
LAYER_DIMS = [(16, 10), (10, 10), (10, 16)]
N_ET = 25
K_SO = 3 * N_ET
PSUM_BANK_F32 = 512


class Cfg:
    def __init__(self, n_nodes, n_edges, npc, tpb):
        self.n_nodes = n_nodes
        self.n_edges = n_edges
        self.npc = npc                       # nodes per core
        self.nblk = math.ceil(npc / 128)     # node blocks per core
        self.npad = 128 * self.nblk
        self.tpb = tpb                       # edge tiles per block (uniform)
        self.epad = self.nblk * tpb * 128    # padded edges per core
        self.ntiles = self.nblk * tpb


_BUILD_CACHE = {}


def build_nc(cfg: Cfg, debug=False):
    import concourse.bacc as bacc
    import concourse.bass as bass
    import concourse.mybir as mybir
    import concourse.tile as tile
    from concourse.masks import make_identity

    f32 = mybir.dt.float32
    bf16 = mybir.dt.bfloat16
    i32 = mybir.dt.int32
    AX = mybir.AxisListType
    OP = mybir.AluOpType

    nc = bacc.Bacc("TRN2", target_bir_lowering=False, debug=False,
                   num_devices=N_CORES)

    nblk, npad, epad, tpb = cfg.nblk, cfg.npad, cfg.epad, cfg.tpb

    # ---- kernel I/O ------------------------------------------------------
    so_d = nc.dram_tensor("so", [K_SO, epad], bf16, kind="ExternalInput")
    st_d = nc.dram_tensor("st", [epad, 128], bf16, kind="ExternalInput")
    xsrc1_d = nc.dram_tensor("xsrc1", [nblk, 128, tpb * 16], bf16,
                             kind="ExternalInput")
    goff_d = nc.dram_tensor("goff", [nblk, 128, tpb], i32,
                            kind="ExternalInput")
    c123_d = [nc.dram_tensor(f"c123_{l}", [K_SO, ic * oc], bf16,
                             kind="ExternalInput")
              for l, (ic, oc) in enumerate(LAYER_DIMS)]
    rb_d = [nc.dram_tensor(f"rb_{l}", [ic, oc], f32, kind="ExternalInput")
            for l, (ic, oc) in enumerate(LAYER_DIMS)]
    bw_d = [nc.dram_tensor(f"bw_{l}", [1, ic * oc], bf16,
                           kind="ExternalInput")
            for l, (ic, oc) in enumerate(LAYER_DIMS)]
    xt1_d = nc.dram_tensor("xt1", [16, npad], f32, kind="ExternalInput")
    degb_d = nc.dram_tensor("degb", [1, npad], bf16, kind="ExternalInput")
    degr_d = nc.dram_tensor("degr", [128, nblk], f32, kind="ExternalInput")
    ppool_d = nc.dram_tensor("ppool", [128, nblk * 8], bf16,
                             kind="ExternalInput")
    cntr_d = nc.dram_tensor("cntr", [8, 1], f32, kind="ExternalInput")
    out_d = nc.dram_tensor("out", [8, 16], f32, kind="ExternalOutput")
    if debug:
        dbg_h1 = nc.dram_tensor("dbg_h1", [128, cfg.nblk * 10], bf16,
                                kind="ExternalOutput")
        dbg_as1 = nc.dram_tensor("dbg_as1", [128, cfg.nblk * 10], f32,
                                 kind="ExternalOutput")
        dbg_rt1 = nc.dram_tensor("dbg_rt1", [128, cfg.nblk * 10], f32,
                                 kind="ExternalOutput")
        dbg_h2 = nc.dram_tensor("dbg_h2", [128, cfg.nblk * 10], bf16,
                                kind="ExternalOutput")
        dbg_rt2 = nc.dram_tensor("dbg_rt2", [128, cfg.nblk * 10], f32,
                                 kind="ExternalOutput")
        dbg_as2 = nc.dram_tensor("dbg_as2", [128, cfg.nblk * 10], f32,
                                 kind="ExternalOutput")
        dbg_xg = nc.dram_tensor("dbg_xg", [128, cfg.tpb * 10], bf16,
                                kind="ExternalOutput")
        dbg_hf = nc.dram_tensor("dbg_hf", [N_CORES, 128, cfg.nblk * 10],
                                bf16, kind="ExternalOutput")

    groups = [list(range(N_CORES))]

    with tile.TileContext(nc) as tc:
        with (
            tc.tile_pool(name="res", bufs=1) as res,
            tc.tile_pool(name="st", bufs=4) as st_pool,
            tc.tile_pool(name="xg", bufs=8) as xg_pool,
            tc.tile_pool(name="pb", bufs=4) as pb_pool,
            tc.tile_pool(name="node", bufs=1) as node_pool,
            tc.tile_pool(name="small", bufs=2) as small_pool,
            tc.tile_pool(name="zp", bufs=2, space="PSUM") as zp_pool,
            tc.tile_pool(name="ag", bufs=2, space="PSUM") as ag_pool,
            tc.tile_pool(name="rt", bufs=1, space="PSUM") as rt_pool,
            tc.tile_pool(name="tp", bufs=2, space="PSUM") as tp_pool,
            tc.tile_pool(name="dram", bufs=1, space="DRAM") as dram,
        ):
            # ---- residents ---------------------------------------------
            so_sb = res.tile([K_SO, epad], bf16)
            nc.sync.dma_start(out=so_sb[:], in_=so_d.ap())
            c123_sb = []
            for l in range(3):
                t = res.tile([K_SO, LAYER_DIMS[l][0] * LAYER_DIMS[l][1]],
                             bf16, tag=f"c123_{l}", name=f"c123s{l}")
                nc.sync.dma_start(out=t[:], in_=c123_d[l].ap())
                c123_sb.append(t)
            rb_sb = []
            bw_sb = []
            for l in range(3):
                ic, oc = LAYER_DIMS[l]
                t = res.tile([ic, oc], f32, tag=f"rb_{l}", name=f"rbs{l}")
                nc.sync.dma_start(out=t[:], in_=rb_d[l].ap())
                rb_sb.append(t)
                t2 = res.tile([1, ic * oc], bf16, tag=f"bw_{l}",
                              name=f"bws{l}")
                nc.sync.dma_start(out=t2[:], in_=bw_d[l].ap())
                bw_sb.append(t2)
            xt1_sb = res.tile([16, npad], f32)
            nc.sync.dma_start(out=xt1_sb[:], in_=xt1_d.ap())
            degb_sb = res.tile([1, npad], bf16)
            nc.sync.dma_start(out=degb_sb[:], in_=degb_d.ap())
            degr_sb = res.tile([128, nblk], f32)
            nc.sync.dma_start(out=degr_sb[:], in_=degr_d.ap())
            ppool_sb = res.tile([128, nblk * 8], bf16)
            nc.sync.dma_start(out=ppool_sb[:], in_=ppool_d.ap())
            cntr_sb = res.tile([8, 1], f32)
            nc.sync.dma_start(out=cntr_sb[:], in_=cntr_d.ap())
            ident = res.tile([128, 128], bf16)
            make_identity(nc, ident[:])
            xt_all = res.tile([10, npad], f32)

            # ---- DRAM scratch ------------------------------------------
            hc = [dram.tile([128, nblk * 10], bf16, tag=f"hc{l}",
                            name=f"hc{l}") for l in range(2)]
            hf = [dram.tile([N_CORES, 128, nblk * 10], bf16, tag=f"hf{l}",
                            name=f"hf{l}", addr_space="Shared")
                  for l in range(2)]
            hfl = [dram.tile([N_CORES, 128, nblk * 10], bf16, tag=f"hfl{l}",
                             name=f"hfl{l}") for l in range(2)]
            pool_in = dram.tile([8, 16], f32)
            pool_out = dram.tile([8, 16], f32, addr_space="Shared")

            for l in range(3):
                ic, oc = LAYER_DIMS[l]
                d = ic * oc
                gmax = PSUM_BANK_F32 // d

                asum = node_pool.tile([128, nblk * oc], f32, tag="asum")
                # ---- edge phase: one batch per node block --------------
                for r in range(nblk):
                    xg = xg_pool.tile([128, tpb, ic], bf16, tag="xg")
                    if l == 0:
                        nc.sync.dma_start(out=xg[:], in_=xsrc1_d.ap()[r])
                    else:
                        xo = xg_pool.tile([128, tpb], i32, tag="xo")
                        nc.sync.dma_start(out=xo[:], in_=goff_d.ap()[r])
                        for t in range(tpb):
                            nc.gpsimd.indirect_dma_start(
                                out=xg[:, t, :].bitcast(f32),
                                out_offset=None,
                                in_=hfl[l - 1][:].rearrange(
                                    "c p (q h) -> (c p q) h",
                                    h=10).bitcast(f32),
                                in_offset=bass.IndirectOffsetOnAxis(
                                    ap=xo[:, t:t + 1], axis=0),
                            )
                    stt = st_pool.tile([128, tpb, 128], bf16, tag="st")
                    nc.sync.dma_start(
                        out=stt[:],
                        in_=st_d.ap()[r * tpb * 128:(r + 1) * tpb * 128, :]
                        .rearrange("(t p) m -> p t m", p=128))
                    if debug and l == 1 and r == 0:
                        nc.sync.dma_start(
                            out=dbg_xg.ap(),
                            in_=xg[:].rearrange("p t i -> p (t i)"))
                    pbuf = pb_pool.tile([128, tpb * d], bf16, tag="pb")
                    t0 = 0
                    while t0 < tpb:
                        g = min(gmax, tpb - t0)
                        zp = zp_pool.tile([128, PSUM_BANK_F32], f32,
                                          tag="zp")
                        for j in range(g):
                            tl = r * tpb + t0 + j
                            nc.tensor.matmul(
                                out=zp[:, j * d:(j + 1) * d],
                                lhsT=so_sb[:, tl * 128:(tl + 1) * 128],
                                rhs=c123_sb[l][:],
                                start=True, stop=True)
                        for j in range(g):
                            t = t0 + j
                            nc.vector.scalar_tensor_tensor(
                                out=pbuf[:, t * d:(t + 1) * d].rearrange(
                                    "p (o i) -> p o i", o=oc, i=ic),
                                in0=zp[:, j * d:(j + 1) * d].rearrange(
                                    "p (o i) -> p o i", o=oc, i=ic),
                                scalar=0.0,
                                in1=xg[:, t:t + 1, :].to_broadcast(
                                    [128, oc, ic]),
                                op0=OP.max, op1=OP.mult)
                        t0 += g
                    # segment-sum into PSUM: bias seed + tpb one-hot matmuls
                    agp = ag_pool.tile([128, d], f32, tag="ag")
                    nc.tensor.matmul(
                        out=agp[:],
                        lhsT=degb_sb[:, r * 128:(r + 1) * 128],
                        rhs=bw_sb[l][:],
                        start=True, stop=False)
                    for t in range(tpb):
                        nc.tensor.matmul(
                            out=agp[:],
                            lhsT=stt[:, t, :],
                            rhs=pbuf[:, t * d:(t + 1) * d],
                            start=False, stop=(t == tpb - 1))
                    nc.vector.reduce_sum(
                        out=asum[:, r * oc:(r + 1) * oc],
                        in_=agp[:].rearrange("p (o i) -> p o i",
                                             o=oc, i=ic),
                        axis=AX.X)

                # ---- node phase (batched over blocks) ------------------
                agg = node_pool.tile([128, nblk * oc], f32, tag="agg")
                nc.vector.tensor_tensor(
                    out=agg[:].rearrange("p (q o) -> p q o", o=oc),
                    in0=asum[:].rearrange("p (q o) -> p q o", o=oc),
                    in1=degr_sb[:].unsqueeze(2).to_broadcast(
                        [128, nblk, oc]),
                    op=OP.mult)
                rt = rt_pool.tile([128, nblk * 16], f32, tag="rt")
                xt = xt1_sb if l == 0 else xt_all
                kr = 16 if l == 0 else 10
                for r in range(nblk):
                    nc.tensor.matmul(
                        out=rt[:, r * oc:(r + 1) * oc],
                        lhsT=xt[0:kr, r * 128:(r + 1) * 128],
                        rhs=rb_sb[l][:],
                        start=True, stop=True)
                hsum = node_pool.tile([128, nblk * oc], f32, tag="hsum")
                nc.vector.scalar_tensor_tensor(
                    out=hsum[:], in0=rt[:, :nblk * oc], scalar=0.0,
                    in1=agg[:], op0=OP.add, op1=OP.add)
                hrel = node_pool.tile([128, nblk * oc], bf16, tag="hrel")
                nc.vector.tensor_scalar_max(out=hrel[:], in0=hsum[:],
                                            scalar1=0.0)

                if debug and l == 1:
                    nc.sync.dma_start(out=dbg_h2.ap(), in_=hrel[:])
                    nc.sync.dma_start(out=dbg_as2.ap(), in_=asum[:])
                    hsf2 = node_pool.tile([128, nblk * oc], f32, tag="hsf2")
                    nc.scalar.copy(out=hsf2[:], in_=rt[:, :nblk * oc])
                    nc.sync.dma_start(out=dbg_rt2.ap(), in_=hsf2[:])
                if debug and l == 0:
                    nc.sync.dma_start(out=dbg_h1.ap(), in_=hrel[:])
                    nc.sync.dma_start(out=dbg_as1.ap(), in_=asum[:])
                    hsf = node_pool.tile([128, nblk * oc], f32, tag="hsf")
                    nc.scalar.copy(out=hsf[:], in_=rt[:, :nblk * oc])
                    nc.sync.dma_start(out=dbg_rt1.ap(), in_=hsf[:])
                if l < 2:
                    nc.sync.dma_start(out=hc[l][:], in_=hrel[:])
                    nc.gpsimd.collective_compute(
                        "AllGather", OP.bypass, replica_groups=groups,
                        ins=[hc[l][:]], outs=[hf[l][:]])
                    nc.sync.dma_start(out=hfl[l][:], in_=hf[l][:])
                    if debug and l == 0:
                        nc.sync.dma_start(out=dbg_hf.ap(), in_=hfl[l][:])
                    for r in range(nblk):
                        tp = tp_pool.tile([16, 128], bf16, tag="tp")
                        nc.tensor.transpose(
                            out=tp[0:oc, :],
                            in_=hrel[:, r * oc:(r + 1) * oc],
                            identity=ident[:])
                        nc.scalar.copy(
                            out=xt_all[0:oc, r * 128:(r + 1) * 128],
                            in_=tp[0:oc, :])
                else:
                    plp = tp_pool.tile([8, 16], f32, tag="tp",
                                       name="plp")
                    for r in range(nblk):
                        nc.tensor.matmul(
                            out=plp[:],
                            lhsT=ppool_sb[:, r * 8:(r + 1) * 8],
                            rhs=hrel[:, r * oc:(r + 1) * oc],
                            start=(r == 0), stop=(r == nblk - 1))
                    pool_sb = small_pool.tile([8, 16], f32, tag="pool_sb")
                    nc.scalar.copy(out=pool_sb[:], in_=plp[:])
                    nc.sync.dma_start(out=pool_in[:], in_=pool_sb[:])
                    nc.gpsimd.collective_compute(
                        "AllReduce", OP.add, replica_groups=groups,
                        ins=[pool_in[:]], outs=[pool_out[:]])
                    pool2 = small_pool.tile([8, 16], f32, tag="pool2")
                    nc.sync.dma_start(out=pool2[:], in_=pool_out[:])
                    outp = small_pool.tile([8, 16], f32, tag="outp")
                    nc.vector.tensor_scalar_mul(out=outp[:], in0=pool2[:],
                                                scalar1=cntr_sb[:, 0:1])
                    nc.sync.dma_start(out=out_d.ap(), in_=outp[:])

    nc.compile()
    return nc


# --------------------------------------------------------------------------
# host-side preparation
# --------------------------------------------------------------------------

def _assign_nodes(deg, n, npc, nblk):
    """LPT bin-packing: nodes -> (core, p, r) slots balancing per-block edge
    load with <=128 nodes per block.  Returns (core, p, r) arrays and the
    max block load."""
    import heapq

    nbins = N_CORES * nblk
    order = np.argsort(-deg, kind="stable")
    heap = [(0, b) for b in range(nbins)]  # (load, bin)
    heapq.heapify(heap)
    counts = np.zeros(nbins, np.int64)
    bin_of = np.zeros(n, np.int64)
    slot_of = np.zeros(n, np.int64)
    spill = []
    for node in order:
        load, b = heapq.heappop(heap)
        bin_of[node] = b
        slot_of[node] = counts[b]
        counts[b] += 1
        load += int(deg[node])
        if counts[b] < 128:
            heapq.heappush(heap, (load, b))
        else:
            spill.append((load, b))
    maxload = max([l for l, _ in heap] + [l for l, _ in spill])
    core = bin_of // nblk
    r = bin_of % nblk
    p = slot_of
    return core, p, r, maxload


def host_prep(cfg: Cfg, inputs: dict):
    np_f32 = np.float32
    bf = ml_dtypes.bfloat16
    x = np.asarray(inputs["x"], np_f32)
    ef = np.asarray(inputs["edge_feat"], np_f32)
    et = np.asarray(inputs["edge_type"]).astype(np.int64)
    src = np.asarray(inputs["edge_src"]).astype(np.int64)
    dst = np.asarray(inputs["edge_dst"]).astype(np.int64)
    cell = np.asarray(inputs["cell_type"]).astype(np.int64)
    bids = np.asarray(inputs["batch_ids"]).astype(np.int64)

    n = cfg.n_nodes
    npc, npad, nblk, tpb = cfg.npc, cfg.npad, cfg.nblk, cfg.tpb
    epad = cfg.epad

    deg = np.bincount(dst, minlength=n).astype(np.int64)
    ncore, npart, nr = inputs["_node_core"], inputs["_node_p"], inputs["_node_r"]

    # folded weights per layer (o-major columns)
    c123, rbs, bws = [], [], []
    for l, (ic, oc) in enumerate(LAYER_DIMS):
        i = l + 1
        emb = np.asarray(inputs[f"emb{i}"], np_f32)
        wh = np.asarray(inputs[f"wh{i}"], np_f32)
        bh = np.asarray(inputs[f"bh{i}"], np_f32)
        wg = np.asarray(inputs[f"wg{i}"], np_f32)
        bg = np.asarray(inputs[f"bg{i}"], np_f32)
        c1 = emb * wh[0][None, :] + wg[0][None, :]
        c2 = emb * wh[1][None, :] + wg[1][None, :]
        c3 = emb * bh[None, :] + bg[None, :]
        m = np.concatenate([c1, c2, c3], axis=0)
        j = np.arange(ic * oc)
        o_, i_ = j // ic, j % ic
        c123.append(np.ascontiguousarray(m[:, i_ * oc + o_]).astype(bf))
        rbs.append(np.asarray(inputs[f"root{i}"], np_f32))
        b = np.asarray(inputs[f"bias{i}"], np_f32)
        bwide = np.zeros(ic * oc, np_f32)
        bwide[np.arange(oc) * ic] = b  # column o*ic + i=0 gets bias[o]
        bws.append(bwide.reshape(1, ic * oc).astype(bf))

    # edge -> (core, block, partition-of-dst)
    e_core = ncore[dst]
    e_r = nr[dst]
    e_p = npart[dst]

    # global h row of src (h layout: row = p*nblk + r per core chunk)
    src_row = (ncore[src] * npad + npart[src] * nblk + nr[src]).astype(np.int64)

    gate = (cell == 1)
    cnt = np.bincount(bids[gate], minlength=8).astype(np_f32)
    cntr = (1.0 / np.maximum(cnt, 1.0)).reshape(8, 1).astype(np_f32)

    in_maps = []
    for c in range(N_CORES):
        esel = np.where(e_core == c)[0]
        # order edges by block, pad each block to tpb*128 slots
        blk = e_r[esel]
        order = np.argsort(blk, kind="stable")
        esel = esel[order]
        blk = blk[order]
        bc = np.bincount(blk, minlength=nblk)
        assert bc.max() <= tpb * 128, (bc.max(), tpb * 128)
        # slot index for each edge
        startslot = np.zeros(nblk, np.int64)
        startslot[:] = np.arange(nblk) * tpb * 128
        within = np.arange(len(esel)) - np.repeat(
            np.concatenate([[0], np.cumsum(bc)[:-1]]), bc)
        slots = np.repeat(startslot, bc) + within

        et_c = et[esel]
        ef_c = ef[esel]

        so = np.zeros((K_SO, epad), np_f32)
        so[et_c, slots] = ef_c[:, 0]
        so[N_ET + et_c, slots] = ef_c[:, 1]
        so[2 * N_ET + et_c, slots] = 1.0

        st = np.zeros((epad, 128), np_f32)
        st[slots, e_p[esel]] = 1.0

        xs = np.zeros((epad, 16), np_f32)
        xs[slots, :] = x[src[esel]]
        xs = xs.reshape(nblk, tpb, 128, 16).transpose(0, 2, 1, 3)
        xs = np.ascontiguousarray(xs).reshape(nblk, 128, tpb * 16)

        gr = np.zeros(epad, np.int64)
        gr[slots] = src_row[esel]
        goff = gr.reshape(nblk, tpb, 128).transpose(0, 2, 1)  # [r, p, t]
        goff = np.ascontiguousarray(goff).astype(np.int32)

        own = np.where(ncore == c)[0]
        ell = npart[own] * nblk + nr[own]

        xt1 = np.zeros((16, npad), np_f32)  # col r*128+p = node (p,r)
        xt1[:, nr[own] * 128 + npart[own]] = x[own].T

        degb = np.zeros((1, npad), np_f32)
        degb[0, nr[own] * 128 + npart[own]] = deg[own]

        degr = np.zeros(npad, np_f32)
        degr[ell] = 1.0 / np.maximum(deg[own], 1.0)
        degr = degr.reshape(128, nblk)

        pp = np.zeros((npad, 8), np_f32)
        g = gate[own]
        pp[ell[g], bids[own][g]] = 1.0
        pp = pp.reshape(128, nblk * 8)

        in_maps.append({
            "so": so.astype(bf),
            "st": st.astype(bf),
            "xsrc1": xs.astype(bf),
            "goff": goff,
            "c123_0": c123[0], "c123_1": c123[1], "c123_2": c123[2],
            "rb_0": rbs[0], "rb_1": rbs[1], "rb_2": rbs[2],
            "bw_0": bws[0], "bw_1": bws[1], "bw_2": bws[2],
            "xt1": xt1,
            "degb": degb.astype(bf),
            "degr": degr.astype(np_f32),
            "ppool": pp.astype(bf),
            "cntr": cntr,
        })
    return in_maps


def prepare(inputs: dict, n_nodes, n_edges, npc):
    """Compute node assignment + cfg (tpb depends on balance)."""
    dst = np.asarray(inputs["edge_dst"]).astype(np.int64)
    deg = np.bincount(dst, minlength=n_nodes).astype(np.int64)
    nblk = math.ceil(npc / 128)
    core, p, r, maxload = _assign_nodes(deg, n_nodes, npc, nblk)
    tpb = max(2, math.ceil(maxload / 128))
    cfg = Cfg(n_nodes, n_edges, npc, tpb)
    inputs = dict(inputs)
    inputs["_node_core"] = core
    inputs["_node_p"] = p
    inputs["_node_r"] = r
    return cfg, inputs


def kernel(**inputs) -> np.ndarray:
    from concourse.bass_utils import run_bass_kernel_spmd

    cfg, inputs2 = prepare(inputs, 50000, 250000, 6250)
    key = ("full", cfg.tpb)
    if key not in _BUILD_CACHE:
        _BUILD_CACHE[key] = build_nc(cfg)
    nc = _BUILD_CACHE[key]
    in_maps = host_prep(cfg, inputs2)
    res = run_bass_kernel_spmd(nc, in_maps, list(range(N_CORES)))
    return np.asarray(res.results[0]["out"], np.float32)
